# revision 9
# baseline (speedup 1.0000x reference)
"""AFT full attention (nn_AFTFullAttention) — 8-core TRN2 Bass kernel.

Sharding: the reference's .view(B,H,T,HD) makes "head" h a block of T/H=256
original time rows per batch reinterpreted as [2048, 128]; one head per core
gives each core complete rows — batch reduction is head-local, out-proj is
row-parallel, no collectives.

v3: fp8e4 DoubleRow matmuls (0.5 cyc/col, K=256/instr) for the AFT numer and
the K projection; bf16 out-projection; fp8 PE transposes (step-2 PSUM) fed
from a contiguous AFT-flat exp(k) store written by the K-evacuation's strided
activation out-AP.  exp(wbias) is precomputed host-side into fp8 (4.2MB vs
16.8MB f32) and DMA'd straight into a resident SBUF store — no on-chip exp
stream.  exp(k) is stored as exp(k-2) (bias folded host-side) so fp8's max
of 240 is never hit; the e^-2 cancels exactly in weighted = num*v/denom.

Phase order K -> transposes -> V -> AFT-numer -> Q -> out-proj keeps the PE
busy end-to-end: the numer matmuls' vector chain needs complete v, so V runs
before it and Q (whose sigmoid evacs aren't needed until the final sq*wsum
multiplies) fills the PE while the chain drains.

Numerics: fp8 only on positive-sum contractions (errors ~delta/sqrt(2048))
and on K inside exp (error averages in the AFT sum); Q/V/out stay bf16.
"""

import os
import sys

sys.path.insert(0, "/opt/trn_rl_repo")

import numpy as np

B, T, DIM, H, HD = 4, 2048, 1024, 8, 128
NCORES = 8
TB = T // H          # 256 original rows per (batch, head-block)
RS = B * TB          # 1024 rows owned per core

KT = DIM // 128      # 8 contraction tiles (dim / c)
ST = T // 128        # 16 s-tiles of the AFT contraction
SP = ST // 2         # 8 DoubleRow s-pairs
TC2 = T // 512       # 4 tau-chunks of 512
RC = RS // 512       # 2 row-chunks of 512
WSCALE = 32.0        # host scales Wk by this to keep fp8 weights ~N(0,1)
KSHIFT = 2.0         # store exp(k - KSHIFT); cancels in weighted/denom

TRACE = False        # set by test.py for profiling runs


def _install_ntff_hook():
    """The agent image's antenv lacks axon_hooks; recreate it so
    run_bass_kernel_spmd(trace=True) can capture NTFF profiles."""
    import types

    try:
        from antenv.axon_hooks import get_axon_ntff_profile_hook  # noqa: F401
        return
    except ImportError:
        pass
    import antenv

    mod = types.ModuleType("antenv.axon_hooks")
    _h = [None]
    mod.set_axon_ntff_profile_hook = lambda h: _h.__setitem__(0, h)
    mod.get_axon_ntff_profile_hook = lambda: _h[0]
    sys.modules["antenv.axon_hooks"] = mod
    antenv.axon_hooks = mod
    from trn_agent_boot.trn_boot import _ntff_profile_via_ctypes

    mod.set_axon_ntff_profile_hook(
        _ntff_profile_via_ctypes("/opt/axon/libaxon_pjrt.so")
    )


def _build():
    import concourse.bacc as bacc
    import concourse.tile as tile
    import concourse.mybir as mybir

    f32 = mybir.dt.float32
    bf16 = mybir.dt.bfloat16
    fp8 = mybir.dt.float8e4
    AF = mybir.ActivationFunctionType
    ALU = mybir.AluOpType
    DR = mybir.MatmulPerfMode.DoubleRow

    nc = bacc.Bacc("TRN2", debug=False, num_devices=NCORES)

    xT = nc.dram_tensor("xT", [128, KT * RS], bf16, kind="ExternalInput")
    xT8 = nc.dram_tensor("xT8", [128, KT * RS], fp8, kind="ExternalInput")
    wqT = nc.dram_tensor("wqT", [128, KT * DIM], bf16, kind="ExternalInput")
    wkT8 = nc.dram_tensor("wkT8", [128, KT * DIM], fp8, kind="ExternalInput")
    wvT = nc.dram_tensor("wvT", [128, KT * DIM], bf16, kind="ExternalInput")
    woT = nc.dram_tensor("woT", [128, KT * DIM], bf16, kind="ExternalInput")
    bq = nc.dram_tensor("bq", [128, KT], f32, kind="ExternalInput")
    bk = nc.dram_tensor("bk", [128, KT], f32, kind="ExternalInput")
    bv = nc.dram_tensor("bv", [128, KT], f32, kind="ExternalInput")
    bo = nc.dram_tensor("bo", [128, KT], f32, kind="ExternalInput")
    # host-precomputed exp(wbias.T) in fp8: [p, st*T + tau], s = st*128+p
    ewtT = nc.dram_tensor("ewtT", [128, ST * T], fp8, kind="ExternalInput")
    ident = nc.dram_tensor("ident", [128, 128], fp8, kind="ExternalInput")
    out = nc.dram_tensor("out", [DIM, RS], f32, kind="ExternalOutput")

    # [c, row] store free-layout: block j (=c//128) at free j*RS + row.
    # AFT view of rows [r0, r0+n): [128(delta), n, 8] with tau = r*8 + j.
    def aft_view(store, r0, n):
        return store.rearrange("p (j r) -> p j r", j=KT)[
            :, :, r0 : r0 + n
        ].transpose([0, 2, 1])

    with tile.TileContext(nc) as tc:
      with (
        tc.tile_pool(name="const", bufs=1) as constp,
        tc.tile_pool(name="pers", bufs=1) as pers,
      ):
        # ---- persistent stores (per-partition bytes in comments) ----
        sq_sb = pers.tile([128, KT * RS], bf16, tag="sq")    # 16K sigmoid(q)->y
        v_tau = pers.tile([128, B * T], f32, tag="v")        # 32K [delta,b*T+tau]
        # exp(k-KSHIFT) in AFT-flat layout [delta(p), b*T + tau] (tau=r*8+j)
        ek_aft = pers.tile([128, B * T], fp8, tag="ek")      # 8K
        ewt_all = pers.tile([128, ST * T], fp8, tag="ewt")   # 32K exp(wbT)
        eks_sb = pers.tile([128, B * T], fp8, tag="eks")     # 8K  [s, b,st,delta]
        wsum = pers.tile([128, T], f32, tag="wsum")          # 8K
        den = pers.tile([128, T], f32, tag="den")            # 8K
        xts = pers.tile([128, KT * RS], bf16, tag="xts")     # 16K
        xts8 = pers.tile([128, KT * RS], fp8, tag="xts8")    # 8K
        wq_sb = pers.tile([128, KT * DIM], bf16, tag="wq")   # 16K
        wk8_sb = pers.tile([128, KT * DIM], fp8, tag="wk8")  # 8K
        wv_sb = pers.tile([128, KT * DIM], bf16, tag="wv")   # 16K

        # ---- t=0 DMA posts ----
        # sync (HW queue): K operands first, then V, then Q.
        nc.sync.dma_start(out=xts8, in_=xT8[:])
        nc.sync.dma_start(out=wk8_sb, in_=wkT8[:])
        nc.sync.dma_start(out=xts, in_=xT[:])
        nc.sync.dma_start(out=wv_sb, in_=wvT[:])
        nc.sync.dma_start(out=wq_sb, in_=wqT[:])
        # gpsimd (SW queue): exp(wbias) fp8, 4 chunks, in parallel with sync.
        for q4 in range(4):
            csz = ST * T // 4
            nc.gpsimd.dma_start(
                out=ewt_all[:, q4 * csz : (q4 + 1) * csz],
                in_=ewtT[:, q4 * csz : (q4 + 1) * csz],
            )
        # scalar (HW queue): small constants, first thing it does.
        id_sb = constp.tile([128, 128], fp8, tag="id")
        nc.scalar.dma_start(out=id_sb, in_=ident[:])
        bias_sb = {}
        for nm, tsr in [("bq", bq), ("bk", bk), ("bv", bv), ("bo", bo)]:
            t_ = constp.tile([128, KT], f32, tag=nm, name=f"b_{nm}")
            nc.scalar.dma_start(out=t_, in_=tsr[:])
            bias_sb[nm] = t_

        ew4 = ewt_all.rearrange("p (st t) -> p st t", st=ST)
        eks4 = eks_sb.rearrange("p (b st d) -> p b st d", b=B, st=ST)
        ek4 = ek_aft.rearrange("p (b r j) -> p b r j", b=B, j=KT)
        xt4 = xts.rearrange("p (rc kt n) -> p rc kt n", rc=RC, kt=KT)
        xt84 = xts8.rearrange("p (rc kt n) -> p rc kt n", rc=RC, kt=KT)
        wq4 = wq_sb.rearrange("p (j kt d) -> p j kt d", j=KT, kt=KT)
        wk84 = wk8_sb.rearrange("p (j kt d) -> p j kt d", j=KT, kt=KT)
        wv4 = wv_sb.rearrange("p (j kt d) -> p j kt d", j=KT, kt=KT)
        v4 = v_tau.rearrange("p (b t j) -> p b t j", b=B, j=KT)

        with (
            tc.tile_pool(name="s3sb", bufs=1) as s3p,
            tc.tile_pool(name="s2f", bufs=1) as s2f,
            tc.tile_pool(name="qkvps", bufs=1, space="PSUM") as ps1,
        ):
            # ---------------- K projection (fp8 DoubleRow) --------------
            for j in range(KT):
                for rc in range(RC):
                    psum = ps1.tile([128, 512], f32, tag="qkv", bufs=3,
                                    name=f"ps_k_{j}_{rc}")
                    for g in range(KT // 2):
                        nc.tensor.matmul(
                            psum,
                            wk84[:, j, 2 * g : 2 * g + 2, :],
                            xt84[:, rc, 2 * g : 2 * g + 2, :],
                            start=(g == 0),
                            stop=(g == KT // 2 - 1),
                            perf_mode=DR,
                        )
                    # exp((psum/WSCALE) + bk - KSHIFT) -> fp8, strided out AP
                    # into the AFT-flat layout (KSHIFT folded into bk host-side)
                    nc.scalar.activation(
                        out=ek4[:, rc * 2 : (rc + 1) * 2, :, j],
                        in_=psum.rearrange("p (b r) -> p b r", b=2),
                        func=AF.Exp,
                        bias=bias_sb["bk"][:, j : j + 1],
                        scale=1.0 / WSCALE,
                    )

            # -------- transposes: ek_aft -> eks (fp8, step-2 psum) ------
            with tc.tile_pool(name="trps", bufs=1, space="PSUM") as pst:
                for idx in range(B * ST):
                    b, st = idx // ST, idx % ST
                    tp = pst.tile([128, 256], fp8, tag="tr", bufs=4,
                                  name=f"tp_{b}_{st}")
                    tp2 = tp.rearrange("p (n two) -> p n two", two=2)[:, :, 0]
                    nc.tensor.transpose(
                        tp2,
                        ek_aft[:, (b * ST + st) * 128 :
                               (b * ST + st + 1) * 128],
                        id_sb,
                    )
                    nc.vector.tensor_copy(out=eks4[:, b, st, :], in_=tp2)

            # ---------------- V projection (bf16) -----------------------
            for j in range(KT):
                for rc in range(RC):
                    psum = ps1.tile([128, 512], f32, tag="qkv", bufs=3,
                                    name=f"ps_v_{j}_{rc}")
                    for kt in range(KT):
                        nc.tensor.matmul(
                            psum,
                            wv4[:, j, kt, :],
                            xt4[:, rc, kt, :],
                            start=(kt == 0),
                            stop=(kt == KT - 1),
                        )
                    nc.vector.tensor_scalar_add(
                        out=v4[:, rc * 2 : (rc + 1) * 2, :, j],
                        in0=psum.rearrange("p (b t) -> p b t", b=2),
                        scalar1=bias_sb["bv"][:, j : j + 1],
                    )

            # prefetch wo tiles on sync queue (landing during stage 2)
            wod_tiles = []
            for dt_ in range(KT):
                wod = s3p.tile([128, KT * 128], bf16, tag="wod", bufs=8,
                               name=f"wod_{dt_}")
                nc.sync.dma_start(
                    out=wod, in_=woT[:, dt_ * KT * 128 : (dt_ + 1) * KT * 128]
                )
                wod_tiles.append(wod)

            # ------------- AFT numer (fp8 DoubleRow) + chain ------------
            with tc.tile_pool(name="s2ps", bufs=1, space="PSUM") as ps2:
                for tc2 in range(TC2):
                    tsl = slice(tc2 * 512, (tc2 + 1) * 512)
                    nps = [ps2.tile([128, 512], f32, tag="np", bufs=5,
                                    name=f"np_{tc2}_{b}") for b in range(B)]
                    for sp in range(SP):
                        for b in range(B):
                            nc.tensor.matmul(
                                nps[b],
                                eks4[:, b, 2 * sp : 2 * sp + 2, :],
                                ew4[:, 2 * sp : 2 * sp + 2, tsl],
                                start=(sp == 0),
                                stop=(sp == SP - 1),
                                perf_mode=DR,
                            )
                    for b in range(B):
                        vview = v_tau[:, b * T + tc2 * 512 :
                                      b * T + (tc2 + 1) * 512]
                        if b == 0:
                            nc.vector.tensor_tensor(
                                out=wsum[:, tsl], in0=nps[b], in1=vview,
                                op=ALU.mult,
                            )
                            nc.vector.tensor_copy(out=den[:, tsl], in_=nps[b])
                        else:
                            nv = s2f.tile([128, 512], f32, tag="nv", bufs=2,
                                          name=f"nv_{tc2}_{b}")
                            nc.vector.tensor_tensor(
                                out=nv, in0=nps[b], in1=vview, op=ALU.mult,
                            )
                            nc.vector.tensor_add(
                                out=wsum[:, tsl], in0=wsum[:, tsl], in1=nv
                            )
                            nc.vector.tensor_add(
                                out=den[:, tsl], in0=den[:, tsl], in1=nps[b]
                            )
                    rec = s2f.tile([128, 512], f32, tag="rec", bufs=2,
                                   name=f"rec_{tc2}")
                    nc.vector.reciprocal_approx_fast(out=rec, in_=den[:, tsl])
                    nc.vector.tensor_tensor(out=wsum[:, tsl], in0=wsum[:, tsl],
                                            in1=rec, op=ALU.mult)

                # ---------------- Q projection (bf16) -------------------
                for j in range(KT):
                    for rc in range(RC):
                        psum = ps1.tile([128, 512], f32, tag="qkv", bufs=3,
                                        name=f"ps_q_{j}_{rc}")
                        for kt in range(KT):
                            nc.tensor.matmul(
                                psum,
                                wq4[:, j, kt, :],
                                xt4[:, rc, kt, :],
                                start=(kt == 0),
                                stop=(kt == KT - 1),
                            )
                        nc.scalar.activation(
                            out=sq_sb[:, j * RS + rc * 512 :
                                      j * RS + (rc + 1) * 512],
                            in_=psum, func=AF.Sigmoid,
                            bias=bias_sb["bq"][:, j : j + 1],
                        )

            # y = sigmoid(q) * weighted (broadcast over b), in AFT view
            for tc2 in range(TC2):
                tsl = slice(tc2 * 512, (tc2 + 1) * 512)
                wgv = wsum[:, tsl].rearrange("p (a c) -> p a c", c=KT)
                for b in range(B):
                    sqv = aft_view(sq_sb, b * TB + tc2 * 64, 64)
                    nc.vector.tensor_tensor(
                        out=sqv, in0=sqv, in1=wgv, op=ALU.mult,
                    )

            # ------------ out projection (bf16), wo resident ------------
            with tc.tile_pool(name="s3ps", bufs=1, space="PSUM") as ps3:
                for dt_ in range(KT):
                    for rc in range(RC):
                        rsl = slice(rc * 512, (rc + 1) * 512)
                        pso = ps3.tile([128, 512], f32, tag="o", bufs=3,
                                       name=f"pso_{rc}_{dt_}")
                        for j in range(KT):
                            nc.tensor.matmul(
                                pso,
                                wod_tiles[dt_][:, j * 128 : (j + 1) * 128],
                                sq_sb[:, j * RS + rc * 512 :
                                      j * RS + (rc + 1) * 512],
                                start=(j == 0),
                                stop=(j == KT - 1),
                            )
                        osb = s3p.tile([128, 512], f32, tag="ot", bufs=3,
                                       name=f"osb_{rc}_{dt_}")
                        nc.scalar.activation(
                            out=osb, in_=pso, func=AF.Identity,
                            bias=bias_sb["bo"][:, dt_ : dt_ + 1],
                        )
                        nc.sync.dma_start(
                            out=out[dt_ * 128 : (dt_ + 1) * 128, rsl],
                            in_=osb,
                        )

    nc.compile()
    return nc


_NC_CACHE = None


def make_in_maps(x, Wq, bq, Wk, bk, Wv, bv, wbias, Wo, bo):
    import ml_dtypes

    f = np.float32
    bf = ml_dtypes.bfloat16
    f8 = ml_dtypes.float8_e4m3
    x = np.asarray(x, f)
    Wq, Wk, Wv, Wo = (np.asarray(a, f) for a in (Wq, Wk, Wv, Wo))
    bq, bk, bv, bo = (np.asarray(a, f) for a in (bq, bk, bv, bo))
    wbias = np.asarray(wbias, f)

    x2 = x.reshape(B * T, DIM)

    def tile_w(W):
        # host[p, X*1024 + Y*128 + d] = W[X*128+d, Y*128+p]
        return np.ascontiguousarray(
            W.reshape(KT, 128, KT, 128).transpose(3, 0, 2, 1).reshape(
                128, KT * KT * 128)
        )

    wqT = tile_w(Wq).astype(bf)
    wkT8 = tile_w(Wk * WSCALE).astype(f8)
    wvT = tile_w(Wv).astype(bf)
    woT = tile_w(Wo).astype(bf)
    id_np = np.eye(128, dtype=f).astype(f8)
    bqc = np.ascontiguousarray(bq.reshape(KT, 128).T)
    bkc = np.ascontiguousarray(bk.reshape(KT, 128).T) - KSHIFT
    bvc = np.ascontiguousarray(bv.reshape(KT, 128).T)
    boc = np.ascontiguousarray(bo.reshape(KT, 128).T)

    in_maps = []
    for c in range(NCORES):
        rows = np.concatenate(
            [x2[b * T + c * TB : b * T + (c + 1) * TB] for b in range(B)]
        )  # [RS, DIM], row = b*TB + t_loc
        xtiled = np.ascontiguousarray(
            rows.T.reshape(KT, 128, RC, 512).transpose(1, 2, 0, 3)
            .reshape(128, KT * RS))
        # ewtT[p, st*T + tau] = exp(wbias[c][tau, st*128+p])
        ewt = np.ascontiguousarray(
            np.exp(wbias[c].T).reshape(ST, 128, T).transpose(1, 0, 2)
            .reshape(128, ST * T)).astype(f8)
        in_maps.append({
            "xT": xtiled.astype(bf),
            "xT8": xtiled.astype(f8),
            "wqT": wqT, "wkT8": wkT8, "wvT": wvT, "woT": woT,
            "bq": bqc, "bk": bkc, "bv": bvc, "bo": boc,
            "ewtT": ewt,
            "ident": id_np,
        })
    return in_maps


def kernel(x, Wq, bq, Wk, bk, Wv, bv, wbias, Wo, bo):
    global _NC_CACHE
    from concourse import bass_utils

    in_maps = make_in_maps(x, Wq, bq, Wk, bk, Wv, bv, wbias, Wo, bo)

    if TRACE:
        _install_ntff_hook()
    if _NC_CACHE is None:
        _NC_CACHE = _build()
    nc = _NC_CACHE

    res = bass_utils.run_bass_kernel_spmd(
        nc, in_maps, core_ids=list(range(NCORES)), trace=TRACE
    )
    f = np.float32
    outf = np.empty((B * T, DIM), f)
    for c in range(NCORES):
        blk = res.results[c]["out"].T  # [RS, DIM], row = b*TB + t_loc
        for b in range(B):
            outf[b * T + c * TB : b * T + (c + 1) * TB] = (
                blk[b * TB : (b + 1) * TB]
            )
    if TRACE:
        kernel.last_exec_time_ns = res.exec_time_ns
        kernel.last_results = res
    return outf.reshape(B, T, DIM)


# revision 14
# speedup vs baseline: 1.4164x; 1.4164x over previous
"""AFT full attention (nn_AFTFullAttention) — 8-core TRN2 Bass kernel.

Sharding: the reference's .view(B,H,T,HD) makes "head" h a block of T/H=256
original time rows per batch reinterpreted as [2048, 128]; one head per core
gives each core complete rows — batch reduction is head-local, out-proj is
row-parallel, no collectives.

v3: fp8e4 DoubleRow matmuls (0.5 cyc/col, K=256/instr) for the AFT numer and
the K projection; bf16 out-projection; fp8 PE transposes (step-2 PSUM) fed
from a contiguous AFT-flat exp(k) store written by the K-evacuation's strided
activation out-AP.  exp(wbias) is precomputed host-side into fp8 (4.2MB vs
16.8MB f32) and DMA'd straight into a resident SBUF store — no on-chip exp
stream.  exp(k) is stored as exp(k-2) (bias folded host-side) so fp8's max
of 240 is never hit; the e^-2 cancels exactly in weighted = num*v/denom.

Phase order K -> transposes -> V -> AFT-numer -> Q -> out-proj keeps the PE
busy end-to-end: the numer matmuls' vector chain needs complete v, so V runs
before it and Q (whose sigmoid evacs aren't needed until the final sq*wsum
multiplies) fills the PE while the chain drains.

Numerics: fp8 only on positive-sum contractions (errors ~delta/sqrt(2048))
and on K inside exp (error averages in the AFT sum); Q/V/out stay bf16.
"""

import os
import sys

sys.path.insert(0, "/opt/trn_rl_repo")

import numpy as np

B, T, DIM, H, HD = 4, 2048, 1024, 8, 128
NCORES = 8
TB = T // H          # 256 original rows per (batch, head-block)
RS = B * TB          # 1024 rows owned per core

KT = DIM // 128      # 8 contraction tiles (dim / c)
ST = T // 128        # 16 s-tiles of the AFT contraction
SP = ST // 2         # 8 DoubleRow s-pairs
TC2 = T // 512       # 4 tau-chunks of 512
RC = RS // 512       # 2 row-chunks of 512
WSCALE = 32.0        # host scales Wk by this to keep fp8 weights ~N(0,1)
KSHIFT = 2.0         # store exp(k - KSHIFT); cancels in weighted/denom

TRACE = False        # set by test.py for profiling runs


def _install_ntff_hook():
    """The agent image's antenv lacks axon_hooks; recreate it so
    run_bass_kernel_spmd(trace=True) can capture NTFF profiles."""
    import types

    try:
        from antenv.axon_hooks import get_axon_ntff_profile_hook  # noqa: F401
        return
    except ImportError:
        pass
    import antenv

    mod = types.ModuleType("antenv.axon_hooks")
    _h = [None]
    mod.set_axon_ntff_profile_hook = lambda h: _h.__setitem__(0, h)
    mod.get_axon_ntff_profile_hook = lambda: _h[0]
    sys.modules["antenv.axon_hooks"] = mod
    antenv.axon_hooks = mod
    from trn_agent_boot.trn_boot import _ntff_profile_via_ctypes

    mod.set_axon_ntff_profile_hook(
        _ntff_profile_via_ctypes("/opt/axon/libaxon_pjrt.so")
    )


def _build():
    import concourse.bacc as bacc
    import concourse.tile as tile
    import concourse.mybir as mybir

    f32 = mybir.dt.float32
    bf16 = mybir.dt.bfloat16
    fp8 = mybir.dt.float8e4
    AF = mybir.ActivationFunctionType
    ALU = mybir.AluOpType
    DR = mybir.MatmulPerfMode.DoubleRow

    nc = bacc.Bacc("TRN2", debug=False, num_devices=NCORES)

    xT = nc.dram_tensor("xT", [128, KT * RS], bf16, kind="ExternalInput")
    xT8 = nc.dram_tensor("xT8", [128, KT * RS], fp8, kind="ExternalInput")
    wqT = nc.dram_tensor("wqT", [128, KT * DIM], bf16, kind="ExternalInput")
    wkT8 = nc.dram_tensor("wkT8", [128, KT * DIM], fp8, kind="ExternalInput")
    wvT = nc.dram_tensor("wvT", [128, KT * DIM], bf16, kind="ExternalInput")
    woT = nc.dram_tensor("woT", [128, KT * DIM], bf16, kind="ExternalInput")
    bq = nc.dram_tensor("bq", [128, KT], f32, kind="ExternalInput")
    bk = nc.dram_tensor("bk", [128, KT], f32, kind="ExternalInput")
    bv = nc.dram_tensor("bv", [128, KT], f32, kind="ExternalInput")
    bo = nc.dram_tensor("bo", [128, KT], f32, kind="ExternalInput")
    # host-precomputed exp(wbias.T) in fp8: [p, st*T + tau], s = st*128+p
    ewtT = nc.dram_tensor("ewtT", [128, ST * T], fp8, kind="ExternalInput")
    ident = nc.dram_tensor("ident", [128, 128], fp8, kind="ExternalInput")
    out = nc.dram_tensor("out", [DIM, RS], f32, kind="ExternalOutput")

    # [c, row] store free-layout: block j (=c//128) at free j*RS + row.
    # AFT view of rows [r0, r0+n): [128(delta), n, 8] with tau = r*8 + j.
    def aft_view(store, r0, n):
        return store.rearrange("p (j r) -> p j r", j=KT)[
            :, :, r0 : r0 + n
        ].transpose([0, 2, 1])

    with tile.TileContext(nc) as tc:
      with (
        tc.tile_pool(name="const", bufs=1) as constp,
        tc.tile_pool(name="pers", bufs=1) as pers,
      ):
        # ---- persistent stores (per-partition bytes in comments) ----
        sq_sb = pers.tile([128, KT * RS], bf16, tag="sq")    # 16K sigmoid(q)->y
        v_tau = pers.tile([128, B * T], f32, tag="v")        # 32K [delta,b*T+tau]
        # exp(k-KSHIFT) in AFT-flat layout [delta(p), b*T + tau] (tau=r*8+j)
        ek_aft = pers.tile([128, B * T], fp8, tag="ek")      # 8K
        ewt_all = pers.tile([128, ST * T], fp8, tag="ewt")   # 32K exp(wbT)
        eks_sb = pers.tile([128, B * T], fp8, tag="eks")     # 8K  [s, b,st,delta]
        wsum = pers.tile([128, T], f32, tag="wsum")          # 8K
        den = pers.tile([128, T], f32, tag="den")            # 8K
        xts = pers.tile([128, KT * RS], bf16, tag="xts")     # 16K
        xts8 = pers.tile([128, KT * RS], fp8, tag="xts8")    # 8K
        wq_sb = pers.tile([128, KT * DIM], bf16, tag="wq")   # 16K
        wk8_sb = pers.tile([128, KT * DIM], fp8, tag="wk8")  # 8K
        wv_sb = pers.tile([128, KT * DIM], bf16, tag="wv")   # 16K

        # ---- t=0 DMA posts ----
        # sync (HW queue): K operands first (split so the first K matmul can
        # start after ~1MB), then V, then Q.
        half = KT * RS // 2
        nc.sync.dma_start(out=xts8[:, :half], in_=xT8[:, :half])
        nc.sync.dma_start(out=wk8_sb[:, : KT * DIM // 2],
                          in_=wkT8[:, : KT * DIM // 2])
        nc.sync.dma_start(out=xts8[:, half:], in_=xT8[:, half:])
        nc.sync.dma_start(out=wk8_sb[:, KT * DIM // 2 :],
                          in_=wkT8[:, KT * DIM // 2 :])
        nc.sync.dma_start(out=xts, in_=xT[:])
        nc.sync.dma_start(out=wv_sb, in_=wvT[:])
        nc.sync.dma_start(out=wq_sb, in_=wqT[:])
        # gpsimd (SW queue): exp(wbias) fp8, 4 chunks, in parallel with sync.
        for q4 in range(4):
            csz = ST * T // 4
            nc.gpsimd.dma_start(
                out=ewt_all[:, q4 * csz : (q4 + 1) * csz],
                in_=ewtT[:, q4 * csz : (q4 + 1) * csz],
            )
        # scalar (HW queue): small constants, first thing it does.
        id_sb = constp.tile([128, 128], fp8, tag="id")
        nc.scalar.dma_start(out=id_sb, in_=ident[:])
        bias_sb = {}
        for nm, tsr in [("bq", bq), ("bk", bk), ("bv", bv), ("bo", bo)]:
            t_ = constp.tile([128, KT], f32, tag=nm, name=f"b_{nm}")
            nc.scalar.dma_start(out=t_, in_=tsr[:])
            bias_sb[nm] = t_

        ew4 = ewt_all.rearrange("p (st t) -> p st t", st=ST)
        eks4 = eks_sb.rearrange("p (b st d) -> p b st d", b=B, st=ST)
        ek4 = ek_aft.rearrange("p (b r j) -> p b r j", b=B, j=KT)
        xt4 = xts.rearrange("p (rc kt n) -> p rc kt n", rc=RC, kt=KT)
        xt84 = xts8.rearrange("p (rc kt n) -> p rc kt n", rc=RC, kt=KT)
        wq4 = wq_sb.rearrange("p (j kt d) -> p j kt d", j=KT, kt=KT)
        wk84 = wk8_sb.rearrange("p (j kt d) -> p j kt d", j=KT, kt=KT)
        wv4 = wv_sb.rearrange("p (j kt d) -> p j kt d", j=KT, kt=KT)
        v4 = v_tau.rearrange("p (b t j) -> p b t j", b=B, j=KT)

        with (
            tc.tile_pool(name="s3sb", bufs=1) as s3p,
            tc.tile_pool(name="s2f", bufs=1) as s2f,
            tc.tile_pool(name="qkvps", bufs=1, space="PSUM") as ps1,
        ):
            # ---------------- K projection (fp8 DoubleRow) --------------
            for j in range(KT):
                for rc in range(RC):
                    psum = ps1.tile([128, 512], f32, tag="qkv", bufs=3,
                                    name=f"ps_k_{j}_{rc}")
                    for g in range(KT // 2):
                        nc.tensor.matmul(
                            psum,
                            wk84[:, j, 2 * g : 2 * g + 2, :],
                            xt84[:, rc, 2 * g : 2 * g + 2, :],
                            start=(g == 0),
                            stop=(g == KT // 2 - 1),
                            perf_mode=DR,
                        )
                    # exp((psum/WSCALE) + bk - KSHIFT) -> fp8, strided out AP
                    # into the AFT-flat layout (KSHIFT folded into bk host-side)
                    nc.scalar.activation(
                        out=ek4[:, rc * 2 : (rc + 1) * 2, :, j],
                        in_=psum.rearrange("p (b r) -> p b r", b=2),
                        func=AF.Exp,
                        bias=bias_sb["bk"][:, j : j + 1],
                        scale=1.0 / WSCALE,
                    )

            # ---------------- V projection (bf16) -----------------------
            for j in range(KT):
                for rc in range(RC):
                    psum = ps1.tile([128, 512], f32, tag="qkv", bufs=3,
                                    name=f"ps_v_{j}_{rc}")
                    for kt in range(KT):
                        nc.tensor.matmul(
                            psum,
                            wv4[:, j, kt, :],
                            xt4[:, rc, kt, :],
                            start=(kt == 0),
                            stop=(kt == KT - 1),
                        )
                    nc.vector.tensor_scalar_add(
                        out=v4[:, rc * 2 : (rc + 1) * 2, :, j],
                        in0=psum.rearrange("p (b t) -> p b t", b=2),
                        scalar1=bias_sb["bv"][:, j : j + 1],
                    )

            # -------- transposes: ek_aft -> eks (fp8, step-2 psum) ------
            # 4 transposes batched per psum tile -> one vector evac per 4.
            with tc.tile_pool(name="trps", bufs=1, space="PSUM") as pst:
                for grp in range(B * ST // 4):
                    b, st0 = grp // 4, (grp % 4) * 4
                    tp = pst.tile([128, 1024], fp8, tag="tr", bufs=3,
                                  name=f"tp_{b}_{st0}")
                    for q in range(4):
                        tq = tp[:, q * 256 : (q + 1) * 256].rearrange(
                            "p (n two) -> p n two", two=2)[:, :, 0]
                        nc.tensor.transpose(
                            tq,
                            ek_aft[:, (b * ST + st0 + q) * 128 :
                                   (b * ST + st0 + q + 1) * 128],
                            id_sb,
                        )
                    nc.vector.tensor_copy(
                        out=eks4[:, b, st0 : st0 + 4, :],
                        in_=tp.rearrange("p (f n two) -> p f n two",
                                         f=4, two=2)[:, :, :, 0],
                    )

            # prefetch wo tiles on sync queue (landing during stage 2)
            wod_tiles = []
            for dt_ in range(KT):
                wod = s3p.tile([128, KT * 128], bf16, tag="wod", bufs=8,
                               name=f"wod_{dt_}")
                nc.sync.dma_start(
                    out=wod, in_=woT[:, dt_ * KT * 128 : (dt_ + 1) * KT * 128]
                )
                wod_tiles.append(wod)

            # ---- AFT numer (fp8 DoubleRow) + chain, Q interleaved ------
            # Q matmuls are emitted between numer chunks so the PE stays
            # busy while each chunk's vector chain drains the nps banks.
            with tc.tile_pool(name="s2ps", bufs=1, space="PSUM") as ps2:
                for tc2 in range(TC2):
                    tsl = slice(tc2 * 512, (tc2 + 1) * 512)
                    nps = [ps2.tile([128, 512], f32, tag="np", bufs=5,
                                    name=f"np_{tc2}_{b}") for b in range(B)]
                    for sp in range(SP):
                        for b in range(B):
                            nc.tensor.matmul(
                                nps[b],
                                eks4[:, b, 2 * sp : 2 * sp + 2, :],
                                ew4[:, 2 * sp : 2 * sp + 2, tsl],
                                start=(sp == 0),
                                stop=(sp == SP - 1),
                                perf_mode=DR,
                            )
                    for b in range(B):
                        vview = v_tau[:, b * T + tc2 * 512 :
                                      b * T + (tc2 + 1) * 512]
                        if b == 0:
                            nc.vector.tensor_tensor(
                                out=wsum[:, tsl], in0=nps[b], in1=vview,
                                op=ALU.mult,
                            )
                            nc.vector.tensor_copy(out=den[:, tsl], in_=nps[b])
                        else:
                            nv = s2f.tile([128, 512], f32, tag="nv", bufs=2,
                                          name=f"nv_{tc2}_{b}")
                            nc.vector.tensor_tensor(
                                out=nv, in0=nps[b], in1=vview, op=ALU.mult,
                            )
                            nc.vector.tensor_add(
                                out=wsum[:, tsl], in0=wsum[:, tsl], in1=nv
                            )
                            nc.vector.tensor_add(
                                out=den[:, tsl], in0=den[:, tsl], in1=nps[b]
                            )
                    rec = s2f.tile([128, 512], f32, tag="rec", bufs=2,
                                   name=f"rec_{tc2}")
                    nc.vector.reciprocal_approx_fast(out=rec, in_=den[:, tsl])
                    nc.vector.tensor_tensor(out=wsum[:, tsl], in0=wsum[:, tsl],
                                            in1=rec, op=ALU.mult)
                    # 4 Q psums per numer chunk (j = 2*tc2, 2*tc2+1)
                    for j in (2 * tc2, 2 * tc2 + 1):
                        for rc in range(RC):
                            psum = ps1.tile([128, 512], f32, tag="qkv",
                                            bufs=3, name=f"ps_q_{j}_{rc}")
                            for kt in range(KT):
                                nc.tensor.matmul(
                                    psum,
                                    wq4[:, j, kt, :],
                                    xt4[:, rc, kt, :],
                                    start=(kt == 0),
                                    stop=(kt == KT - 1),
                                )
                            nc.scalar.activation(
                                out=sq_sb[:, j * RS + rc * 512 :
                                          j * RS + (rc + 1) * 512],
                                in_=psum, func=AF.Sigmoid,
                                bias=bias_sb["bq"][:, j : j + 1],
                            )

            # y = sigmoid(q) * weighted, in the [c,row] layout: for block j,
            # in1[row] = wsum[p, t*8 + j] (t = row % 256), contiguous out.
            for j in range(KT):
                wsl = wsum.rearrange("p (t j) -> p t j", j=KT)[:, :, j]
                for rc in range(RC):
                    for u in range(2):
                        o0 = j * RS + rc * 512 + u * 256
                        nc.vector.tensor_tensor(
                            out=sq_sb[:, o0 : o0 + 256],
                            in0=sq_sb[:, o0 : o0 + 256],
                            in1=wsl, op=ALU.mult,
                        )

            # ------------ out projection (bf16), wo resident ------------
            with tc.tile_pool(name="s3ps", bufs=1, space="PSUM") as ps3:
                for dt_ in range(KT):
                    for rc in range(RC):
                        rsl = slice(rc * 512, (rc + 1) * 512)
                        pso = ps3.tile([128, 512], f32, tag="o", bufs=3,
                                       name=f"pso_{rc}_{dt_}")
                        for j in range(KT):
                            nc.tensor.matmul(
                                pso,
                                wod_tiles[dt_][:, j * 128 : (j + 1) * 128],
                                sq_sb[:, j * RS + rc * 512 :
                                      j * RS + (rc + 1) * 512],
                                start=(j == 0),
                                stop=(j == KT - 1),
                            )
                        osb = s3p.tile([128, 512], f32, tag="ot", bufs=3,
                                       name=f"osb_{rc}_{dt_}")
                        nc.scalar.activation(
                            out=osb, in_=pso, func=AF.Identity,
                            bias=bias_sb["bo"][:, dt_ : dt_ + 1],
                        )
                        nc.sync.dma_start(
                            out=out[dt_ * 128 : (dt_ + 1) * 128, rsl],
                            in_=osb,
                        )

    nc.compile()
    return nc


_NC_CACHE = None


def make_in_maps(x, Wq, bq, Wk, bk, Wv, bv, wbias, Wo, bo):
    import ml_dtypes

    f = np.float32
    bf = ml_dtypes.bfloat16
    f8 = ml_dtypes.float8_e4m3
    x = np.asarray(x, f)
    Wq, Wk, Wv, Wo = (np.asarray(a, f) for a in (Wq, Wk, Wv, Wo))
    bq, bk, bv, bo = (np.asarray(a, f) for a in (bq, bk, bv, bo))
    wbias = np.asarray(wbias, f)

    x2 = x.reshape(B * T, DIM)

    def tile_w(W):
        # host[p, X*1024 + Y*128 + d] = W[X*128+d, Y*128+p]
        return np.ascontiguousarray(
            W.reshape(KT, 128, KT, 128).transpose(3, 0, 2, 1).reshape(
                128, KT * KT * 128)
        )

    wqT = tile_w(Wq).astype(bf)
    wkT8 = tile_w(Wk * WSCALE).astype(f8)
    wvT = tile_w(Wv).astype(bf)
    woT = tile_w(Wo).astype(bf)
    id_np = np.eye(128, dtype=f).astype(f8)
    bqc = np.ascontiguousarray(bq.reshape(KT, 128).T)
    bkc = np.ascontiguousarray(bk.reshape(KT, 128).T) - KSHIFT
    bvc = np.ascontiguousarray(bv.reshape(KT, 128).T)
    boc = np.ascontiguousarray(bo.reshape(KT, 128).T)

    in_maps = []
    for c in range(NCORES):
        rows = np.concatenate(
            [x2[b * T + c * TB : b * T + (c + 1) * TB] for b in range(B)]
        )  # [RS, DIM], row = b*TB + t_loc
        xtiled = np.ascontiguousarray(
            rows.T.reshape(KT, 128, RC, 512).transpose(1, 2, 0, 3)
            .reshape(128, KT * RS))
        # ewtT[p, st*T + tau] = exp(wbias[c][tau, st*128+p])
        ewt = np.ascontiguousarray(
            np.exp(wbias[c].T).reshape(ST, 128, T).transpose(1, 0, 2)
            .reshape(128, ST * T)).astype(f8)
        in_maps.append({
            "xT": xtiled.astype(bf),
            "xT8": xtiled.astype(f8),
            "wqT": wqT, "wkT8": wkT8, "wvT": wvT, "woT": woT,
            "bq": bqc, "bk": bkc, "bv": bvc, "bo": boc,
            "ewtT": ewt,
            "ident": id_np,
        })
    return in_maps


def kernel(x, Wq, bq, Wk, bk, Wv, bv, wbias, Wo, bo):
    global _NC_CACHE
    from concourse import bass_utils

    in_maps = make_in_maps(x, Wq, bq, Wk, bk, Wv, bv, wbias, Wo, bo)

    if TRACE:
        _install_ntff_hook()
    if _NC_CACHE is None:
        _NC_CACHE = _build()
    nc = _NC_CACHE

    res = bass_utils.run_bass_kernel_spmd(
        nc, in_maps, core_ids=list(range(NCORES)), trace=TRACE
    )
    f = np.float32
    outf = np.empty((B * T, DIM), f)
    for c in range(NCORES):
        blk = res.results[c]["out"].T  # [RS, DIM], row = b*TB + t_loc
        for b in range(B):
            outf[b * T + c * TB : b * T + (c + 1) * TB] = (
                blk[b * TB : (b + 1) * TB]
            )
    if TRACE:
        kernel.last_exec_time_ns = res.exec_time_ns
        kernel.last_results = res
    return outf.reshape(B, T, DIM)


# revision 18
# speedup vs baseline: 1.4290x; 1.0089x over previous
"""AFT full attention (nn_AFTFullAttention) — 8-core TRN2 Bass kernel.

Sharding: the reference's .view(B,H,T,HD) makes "head" h a block of T/H=256
original time rows per batch reinterpreted as [2048, 128]; one head per core
gives each core complete rows — batch reduction is head-local, out-proj is
row-parallel, no collectives.

v3: fp8e4 DoubleRow matmuls (0.5 cyc/col, K=256/instr) for the AFT numer and
the K projection; bf16 out-projection; fp8 PE transposes (step-2 PSUM) fed
from a contiguous AFT-flat exp(k) store written by the K-evacuation's strided
activation out-AP.  exp(wbias) is precomputed host-side into fp8 (4.2MB vs
16.8MB f32) and DMA'd straight into a resident SBUF store — no on-chip exp
stream.  exp(k) is stored as exp(k-2) (bias folded host-side) so fp8's max
of 240 is never hit; the e^-2 cancels exactly in weighted = num*v/denom.

Phase order K -> transposes -> V -> AFT-numer -> Q -> out-proj keeps the PE
busy end-to-end: the numer matmuls' vector chain needs complete v, so V runs
before it and Q (whose sigmoid evacs aren't needed until the final sq*wsum
multiplies) fills the PE while the chain drains.

Numerics: fp8 only on positive-sum contractions (errors ~delta/sqrt(2048))
and on K inside exp (error averages in the AFT sum); Q/V/out stay bf16.
"""

import os
import sys

sys.path.insert(0, "/opt/trn_rl_repo")

import numpy as np

B, T, DIM, H, HD = 4, 2048, 1024, 8, 128
NCORES = 8
TB = T // H          # 256 original rows per (batch, head-block)
RS = B * TB          # 1024 rows owned per core

KT = DIM // 128      # 8 contraction tiles (dim / c)
ST = T // 128        # 16 s-tiles of the AFT contraction
SP = ST // 2         # 8 DoubleRow s-pairs
TC2 = T // 512       # 4 tau-chunks of 512
RC = RS // 512       # 2 row-chunks of 512
WSCALE = 32.0        # host scales Wk by this to keep fp8 weights ~N(0,1)
KSHIFT = 2.0         # store exp(k - KSHIFT); cancels in weighted/denom

TRACE = False        # set by test.py for profiling runs


def _install_ntff_hook():
    """The agent image's antenv lacks axon_hooks; recreate it so
    run_bass_kernel_spmd(trace=True) can capture NTFF profiles."""
    import types

    try:
        from antenv.axon_hooks import get_axon_ntff_profile_hook  # noqa: F401
        return
    except ImportError:
        pass
    import antenv

    mod = types.ModuleType("antenv.axon_hooks")
    _h = [None]
    mod.set_axon_ntff_profile_hook = lambda h: _h.__setitem__(0, h)
    mod.get_axon_ntff_profile_hook = lambda: _h[0]
    sys.modules["antenv.axon_hooks"] = mod
    antenv.axon_hooks = mod
    from trn_agent_boot.trn_boot import _ntff_profile_via_ctypes

    mod.set_axon_ntff_profile_hook(
        _ntff_profile_via_ctypes("/opt/axon/libaxon_pjrt.so")
    )


def _build():
    import concourse.bacc as bacc
    import concourse.tile as tile
    import concourse.mybir as mybir

    f32 = mybir.dt.float32
    bf16 = mybir.dt.bfloat16
    fp8 = mybir.dt.float8e4
    AF = mybir.ActivationFunctionType
    ALU = mybir.AluOpType
    DR = mybir.MatmulPerfMode.DoubleRow

    nc = bacc.Bacc("TRN2", debug=False, num_devices=NCORES)

    xT = nc.dram_tensor("xT", [128, KT * RS], bf16, kind="ExternalInput")
    xT8 = nc.dram_tensor("xT8", [128, KT * RS], fp8, kind="ExternalInput")
    wqT = nc.dram_tensor("wqT", [128, KT * DIM], bf16, kind="ExternalInput")
    wkT8 = nc.dram_tensor("wkT8", [128, KT * DIM], fp8, kind="ExternalInput")
    wvT = nc.dram_tensor("wvT", [128, KT * DIM], bf16, kind="ExternalInput")
    woT = nc.dram_tensor("woT", [128, KT * DIM], bf16, kind="ExternalInput")
    bq = nc.dram_tensor("bq", [128, KT], f32, kind="ExternalInput")
    bk = nc.dram_tensor("bk", [128, KT], f32, kind="ExternalInput")
    bv = nc.dram_tensor("bv", [128, KT], f32, kind="ExternalInput")
    bo = nc.dram_tensor("bo", [128, KT], f32, kind="ExternalInput")
    # host-precomputed exp(wbias.T) in fp8: [p, st*T + tau], s = st*128+p
    ewtT = nc.dram_tensor("ewtT", [128, ST * T], fp8, kind="ExternalInput")
    ident = nc.dram_tensor("ident", [128, 128], fp8, kind="ExternalInput")
    out = nc.dram_tensor("out", [DIM, RS], f32, kind="ExternalOutput")

    # [c, row] store free-layout: block j (=c//128) at free j*RS + row.
    # AFT view of rows [r0, r0+n): [128(delta), n, 8] with tau = r*8 + j.
    def aft_view(store, r0, n):
        return store.rearrange("p (j r) -> p j r", j=KT)[
            :, :, r0 : r0 + n
        ].transpose([0, 2, 1])

    with tile.TileContext(nc) as tc:
      with (
        tc.tile_pool(name="const", bufs=1) as constp,
        tc.tile_pool(name="pers", bufs=1) as pers,
      ):
        # ---- persistent stores (per-partition bytes in comments) ----
        sq_sb = pers.tile([128, KT * RS], bf16, tag="sq")    # 16K sigmoid(q)->y
        v_tau = pers.tile([128, B * T], f32, tag="v")        # 32K [delta,b*T+tau]
        # exp(k-KSHIFT) in AFT-flat layout [delta(p), b*T + tau] (tau=r*8+j)
        ek_aft = pers.tile([128, B * T], fp8, tag="ek")      # 8K
        ewt_all = pers.tile([128, ST * T], fp8, tag="ewt")   # 32K exp(wbT)
        eks_sb = pers.tile([128, B * T], fp8, tag="eks")     # 8K  [s, b,st,delta]
        wsum = pers.tile([128, T], f32, tag="wsum")          # 8K
        den = pers.tile([128, T], f32, tag="den")            # 8K
        xts = pers.tile([128, KT * RS], bf16, tag="xts")     # 16K
        xts8 = pers.tile([128, KT * RS], fp8, tag="xts8")    # 8K
        wq_sb = pers.tile([128, KT * DIM], bf16, tag="wq")   # 16K
        wk8_sb = pers.tile([128, KT * DIM], fp8, tag="wk8")  # 8K
        wv_sb = pers.tile([128, KT * DIM], bf16, tag="wv")   # 16K

        # ---- t=0 DMA posts ----
        # sync (HW queue): K operands first, finely split so the first K
        # matmul starts after ~384KB, then V, then Q.
        half = KT * RS // 2
        quarter = half // 2
        nc.sync.dma_start(out=wk8_sb[:, : KT * 128],
                          in_=wkT8[:, : KT * 128])
        nc.sync.dma_start(out=xts8[:, :quarter], in_=xT8[:, :quarter])
        nc.sync.dma_start(out=xts8[:, quarter:half],
                          in_=xT8[:, quarter:half])
        nc.sync.dma_start(out=wk8_sb[:, KT * 128 :], in_=wkT8[:, KT * 128 :])
        nc.sync.dma_start(out=xts8[:, half:], in_=xT8[:, half:])
        nc.sync.dma_start(out=xts, in_=xT[:])
        nc.sync.dma_start(out=wv_sb, in_=wvT[:])
        nc.sync.dma_start(out=wq_sb, in_=wqT[:])
        # gpsimd (SW queue): exp(wbias) fp8, 4 chunks, in parallel with sync.
        for q4 in range(4):
            csz = ST * T // 4
            nc.gpsimd.dma_start(
                out=ewt_all[:, q4 * csz : (q4 + 1) * csz],
                in_=ewtT[:, q4 * csz : (q4 + 1) * csz],
            )
        # scalar (HW queue): small constants, first thing it does.
        id_sb = constp.tile([128, 128], fp8, tag="id")
        nc.scalar.dma_start(out=id_sb, in_=ident[:])
        bias_sb = {}
        for nm, tsr in [("bq", bq), ("bk", bk), ("bv", bv), ("bo", bo)]:
            t_ = constp.tile([128, KT], f32, tag=nm, name=f"b_{nm}")
            nc.scalar.dma_start(out=t_, in_=tsr[:])
            bias_sb[nm] = t_

        ew4 = ewt_all.rearrange("p (st t) -> p st t", st=ST)
        eks4 = eks_sb.rearrange("p (b st d) -> p b st d", b=B, st=ST)
        ek4 = ek_aft.rearrange("p (b r j) -> p b r j", b=B, j=KT)
        xt4 = xts.rearrange("p (rc kt n) -> p rc kt n", rc=RC, kt=KT)
        xt84 = xts8.rearrange("p (rc kt n) -> p rc kt n", rc=RC, kt=KT)
        wq4 = wq_sb.rearrange("p (j kt d) -> p j kt d", j=KT, kt=KT)
        wk84 = wk8_sb.rearrange("p (j kt d) -> p j kt d", j=KT, kt=KT)
        wv4 = wv_sb.rearrange("p (j kt d) -> p j kt d", j=KT, kt=KT)
        v4 = v_tau.rearrange("p (b t j) -> p b t j", b=B, j=KT)

        with (
            tc.tile_pool(name="s3sb", bufs=1) as s3p,
            tc.tile_pool(name="s2f", bufs=1) as s2f,
            tc.tile_pool(name="qkvps", bufs=1, space="PSUM") as ps1,
        ):
            # ---------------- K projection (fp8 DoubleRow) --------------
            for j in range(KT):
                for rc in range(RC):
                    psum = ps1.tile([128, 512], f32, tag="qkv", bufs=3,
                                    name=f"ps_k_{j}_{rc}")
                    for g in range(KT // 2):
                        nc.tensor.matmul(
                            psum,
                            wk84[:, j, 2 * g : 2 * g + 2, :],
                            xt84[:, rc, 2 * g : 2 * g + 2, :],
                            start=(g == 0),
                            stop=(g == KT // 2 - 1),
                            perf_mode=DR,
                        )
                    # exp((psum/WSCALE) + bk - KSHIFT) -> fp8 contiguous on
                    # scalar, then vector scatters into the AFT-flat layout
                    # (strided writes are ~2x slower; splitting engines
                    # keeps the K psum recycling off the scalar's back)
                    ekc = s2f.tile([128, 512], fp8, tag="ekc", bufs=3,
                                   name=f"ekc_{j}_{rc}")
                    nc.scalar.activation(
                        out=ekc, in_=psum, func=AF.Exp,
                        bias=bias_sb["bk"][:, j : j + 1],
                        scale=1.0 / WSCALE,
                    )
                    nc.vector.tensor_copy(
                        out=ek4[:, rc * 2 : (rc + 1) * 2, :, j],
                        in_=ekc.rearrange("p (b r) -> p b r", b=2),
                    )

            # ---------------- V projection (bf16) -----------------------
            for j in range(KT):
                for rc in range(RC):
                    psum = ps1.tile([128, 512], f32, tag="qkv", bufs=3,
                                    name=f"ps_v_{j}_{rc}")
                    for kt in range(KT):
                        nc.tensor.matmul(
                            psum,
                            wv4[:, j, kt, :],
                            xt4[:, rc, kt, :],
                            start=(kt == 0),
                            stop=(kt == KT - 1),
                        )
                    nc.vector.tensor_scalar_add(
                        out=v4[:, rc * 2 : (rc + 1) * 2, :, j],
                        in0=psum.rearrange("p (b t) -> p b t", b=2),
                        scalar1=bias_sb["bv"][:, j : j + 1],
                    )

            # -------- transposes: ek_aft -> eks (fp8, step-2 psum) ------
            # 4 transposes batched per psum tile -> one vector evac per 4.
            with tc.tile_pool(name="trps", bufs=1, space="PSUM") as pst:
                for grp in range(B * ST // 4):
                    b, st0 = grp // 4, (grp % 4) * 4
                    tp = pst.tile([128, 1024], fp8, tag="tr", bufs=3,
                                  name=f"tp_{b}_{st0}")
                    for q in range(4):
                        tq = tp[:, q * 256 : (q + 1) * 256].rearrange(
                            "p (n two) -> p n two", two=2)[:, :, 0]
                        nc.tensor.transpose(
                            tq,
                            ek_aft[:, (b * ST + st0 + q) * 128 :
                                   (b * ST + st0 + q + 1) * 128],
                            id_sb,
                        )
                    nc.vector.tensor_copy(
                        out=eks4[:, b, st0 : st0 + 4, :],
                        in_=tp.rearrange("p (f n two) -> p f n two",
                                         f=4, two=2)[:, :, :, 0],
                    )

            # prefetch wo tiles on sync queue (landing during stage 2)
            wod_tiles = []
            for dt_ in range(KT):
                wod = s3p.tile([128, KT * 128], bf16, tag="wod", bufs=8,
                               name=f"wod_{dt_}")
                nc.sync.dma_start(
                    out=wod, in_=woT[:, dt_ * KT * 128 : (dt_ + 1) * KT * 128]
                )
                wod_tiles.append(wod)

            # ---- AFT numer (fp8 DoubleRow) + chain, Q interleaved ------
            # Q matmuls are emitted between numer chunks so the PE stays
            # busy while each chunk's vector chain drains the nps banks.
            with tc.tile_pool(name="s2ps", bufs=1, space="PSUM") as ps2:
                for tc2 in range(TC2):
                    tsl = slice(tc2 * 512, (tc2 + 1) * 512)
                    nps = [ps2.tile([128, 512], f32, tag="np", bufs=5,
                                    name=f"np_{tc2}_{b}") for b in range(B)]
                    for sp in range(SP):
                        for b in range(B):
                            nc.tensor.matmul(
                                nps[b],
                                eks4[:, b, 2 * sp : 2 * sp + 2, :],
                                ew4[:, 2 * sp : 2 * sp + 2, tsl],
                                start=(sp == 0),
                                stop=(sp == SP - 1),
                                perf_mode=DR,
                            )
                    for b in range(B):
                        vview = v_tau[:, b * T + tc2 * 512 :
                                      b * T + (tc2 + 1) * 512]
                        if b == 0:
                            nc.vector.tensor_tensor(
                                out=wsum[:, tsl], in0=nps[b], in1=vview,
                                op=ALU.mult,
                            )
                            nc.vector.tensor_copy(out=den[:, tsl], in_=nps[b])
                        else:
                            nv = s2f.tile([128, 512], f32, tag="nv", bufs=2,
                                          name=f"nv_{tc2}_{b}")
                            nc.vector.tensor_tensor(
                                out=nv, in0=nps[b], in1=vview, op=ALU.mult,
                            )
                            nc.vector.tensor_add(
                                out=wsum[:, tsl], in0=wsum[:, tsl], in1=nv
                            )
                            nc.vector.tensor_add(
                                out=den[:, tsl], in0=den[:, tsl], in1=nps[b]
                            )
                    rec = s2f.tile([128, 512], f32, tag="rec", bufs=2,
                                   name=f"rec_{tc2}")
                    nc.vector.reciprocal_approx_fast(out=rec, in_=den[:, tsl])
                    nc.vector.tensor_tensor(out=wsum[:, tsl], in0=wsum[:, tsl],
                                            in1=rec, op=ALU.mult)
                    # 4 Q psums per numer chunk (j = 2*tc2, 2*tc2+1)
                    for j in (2 * tc2, 2 * tc2 + 1):
                        for rc in range(RC):
                            psum = ps1.tile([128, 512], f32, tag="qkv",
                                            bufs=3, name=f"ps_q_{j}_{rc}")
                            for kt in range(KT):
                                nc.tensor.matmul(
                                    psum,
                                    wq4[:, j, kt, :],
                                    xt4[:, rc, kt, :],
                                    start=(kt == 0),
                                    stop=(kt == KT - 1),
                                )
                            nc.scalar.activation(
                                out=sq_sb[:, j * RS + rc * 512 :
                                          j * RS + (rc + 1) * 512],
                                in_=psum, func=AF.Sigmoid,
                                bias=bias_sb["bq"][:, j : j + 1],
                            )

            # y = sigmoid(q) * weighted, in the [c,row] layout: for block j,
            # in1[row] = wsum[p, t*8 + j] (t = row % 256), contiguous out.
            for j in range(KT):
                wsl = wsum.rearrange("p (t j) -> p t j", j=KT)[:, :, j]
                for rc in range(RC):
                    for u in range(2):
                        o0 = j * RS + rc * 512 + u * 256
                        nc.vector.tensor_tensor(
                            out=sq_sb[:, o0 : o0 + 256],
                            in0=sq_sb[:, o0 : o0 + 256],
                            in1=wsl, op=ALU.mult,
                        )

            # ------------ out projection (bf16), wo resident ------------
            # evac alternates scalar/vector and the out-DMA alternates
            # sync/scalar queues so no single engine paces the drain.
            with tc.tile_pool(name="s3ps", bufs=1, space="PSUM") as ps3:
                for dt_ in range(KT):
                    for rc in range(RC):
                        i3 = dt_ * RC + rc
                        rsl = slice(rc * 512, (rc + 1) * 512)
                        pso = ps3.tile([128, 512], f32, tag="o", bufs=5,
                                       name=f"pso_{rc}_{dt_}")
                        for j in range(KT):
                            nc.tensor.matmul(
                                pso,
                                wod_tiles[dt_][:, j * 128 : (j + 1) * 128],
                                sq_sb[:, j * RS + rc * 512 :
                                      j * RS + (rc + 1) * 512],
                                start=(j == 0),
                                stop=(j == KT - 1),
                            )
                        osb = s3p.tile([128, 512], f32, tag="ot", bufs=3,
                                       name=f"osb_{rc}_{dt_}")
                        if i3 % 2 == 0:
                            nc.scalar.activation(
                                out=osb, in_=pso, func=AF.Identity,
                                bias=bias_sb["bo"][:, dt_ : dt_ + 1],
                            )
                            nc.sync.dma_start(
                                out=out[dt_ * 128 : (dt_ + 1) * 128, rsl],
                                in_=osb,
                            )
                        else:
                            nc.vector.tensor_scalar_add(
                                out=osb, in0=pso,
                                scalar1=bias_sb["bo"][:, dt_ : dt_ + 1],
                            )
                            nc.scalar.dma_start(
                                out=out[dt_ * 128 : (dt_ + 1) * 128, rsl],
                                in_=osb,
                            )

    nc.compile()
    return nc


_NC_CACHE = None


def make_in_maps(x, Wq, bq, Wk, bk, Wv, bv, wbias, Wo, bo):
    import ml_dtypes

    f = np.float32
    bf = ml_dtypes.bfloat16
    f8 = ml_dtypes.float8_e4m3
    x = np.asarray(x, f)
    Wq, Wk, Wv, Wo = (np.asarray(a, f) for a in (Wq, Wk, Wv, Wo))
    bq, bk, bv, bo = (np.asarray(a, f) for a in (bq, bk, bv, bo))
    wbias = np.asarray(wbias, f)

    x2 = x.reshape(B * T, DIM)

    def tile_w(W):
        # host[p, X*1024 + Y*128 + d] = W[X*128+d, Y*128+p]
        return np.ascontiguousarray(
            W.reshape(KT, 128, KT, 128).transpose(3, 0, 2, 1).reshape(
                128, KT * KT * 128)
        )

    wqT = tile_w(Wq).astype(bf)
    wkT8 = tile_w(Wk * WSCALE).astype(f8)
    wvT = tile_w(Wv).astype(bf)
    woT = tile_w(Wo).astype(bf)
    id_np = np.eye(128, dtype=f).astype(f8)
    bqc = np.ascontiguousarray(bq.reshape(KT, 128).T)
    bkc = np.ascontiguousarray(bk.reshape(KT, 128).T) - KSHIFT
    bvc = np.ascontiguousarray(bv.reshape(KT, 128).T)
    boc = np.ascontiguousarray(bo.reshape(KT, 128).T)

    in_maps = []
    for c in range(NCORES):
        rows = np.concatenate(
            [x2[b * T + c * TB : b * T + (c + 1) * TB] for b in range(B)]
        )  # [RS, DIM], row = b*TB + t_loc
        xtiled = np.ascontiguousarray(
            rows.T.reshape(KT, 128, RC, 512).transpose(1, 2, 0, 3)
            .reshape(128, KT * RS))
        # ewtT[p, st*T + tau] = exp(wbias[c][tau, st*128+p])
        ewt = np.ascontiguousarray(
            np.exp(wbias[c].T).reshape(ST, 128, T).transpose(1, 0, 2)
            .reshape(128, ST * T)).astype(f8)
        in_maps.append({
            "xT": xtiled.astype(bf),
            "xT8": xtiled.astype(f8),
            "wqT": wqT, "wkT8": wkT8, "wvT": wvT, "woT": woT,
            "bq": bqc, "bk": bkc, "bv": bvc, "bo": boc,
            "ewtT": ewt,
            "ident": id_np,
        })
    return in_maps


def kernel(x, Wq, bq, Wk, bk, Wv, bv, wbias, Wo, bo):
    global _NC_CACHE
    from concourse import bass_utils

    in_maps = make_in_maps(x, Wq, bq, Wk, bk, Wv, bv, wbias, Wo, bo)

    if TRACE:
        _install_ntff_hook()
    if _NC_CACHE is None:
        _NC_CACHE = _build()
    nc = _NC_CACHE

    res = bass_utils.run_bass_kernel_spmd(
        nc, in_maps, core_ids=list(range(NCORES)), trace=TRACE
    )
    f = np.float32
    outf = np.empty((B * T, DIM), f)
    for c in range(NCORES):
        blk = res.results[c]["out"].T  # [RS, DIM], row = b*TB + t_loc
        for b in range(B):
            outf[b * T + c * TB : b * T + (c + 1) * TB] = (
                blk[b * TB : (b + 1) * TB]
            )
    if TRACE:
        kernel.last_exec_time_ns = res.exec_time_ns
        kernel.last_results = res
    return outf.reshape(B, T, DIM)


# revision 19
# speedup vs baseline: 1.5356x; 1.0746x over previous
"""AFT full attention (nn_AFTFullAttention) — 8-core TRN2 Bass kernel.

Sharding: the reference's .view(B,H,T,HD) makes "head" h a block of T/H=256
original time rows per batch reinterpreted as [2048, 128]; one head per core
gives each core complete rows — batch reduction is head-local, out-proj is
row-parallel, no collectives.

v3: fp8e4 DoubleRow matmuls (0.5 cyc/col, K=256/instr) for the AFT numer and
the K projection; bf16 out-projection; fp8 PE transposes (step-2 PSUM) fed
from a contiguous AFT-flat exp(k) store written by the K-evacuation's strided
activation out-AP.  exp(wbias) is precomputed host-side into fp8 (4.2MB vs
16.8MB f32) and DMA'd straight into a resident SBUF store — no on-chip exp
stream.  exp(k) is stored as exp(k-2) (bias folded host-side) so fp8's max
of 240 is never hit; the e^-2 cancels exactly in weighted = num*v/denom.

Phase order K -> transposes -> V -> AFT-numer -> Q -> out-proj keeps the PE
busy end-to-end: the numer matmuls' vector chain needs complete v, so V runs
before it and Q (whose sigmoid evacs aren't needed until the final sq*wsum
multiplies) fills the PE while the chain drains.

Numerics: fp8 only on positive-sum contractions (errors ~delta/sqrt(2048))
and on K inside exp (error averages in the AFT sum); Q/V/out stay bf16.
"""

import os
import sys

sys.path.insert(0, "/opt/trn_rl_repo")

import numpy as np

B, T, DIM, H, HD = 4, 2048, 1024, 8, 128
NCORES = 8
TB = T // H          # 256 original rows per (batch, head-block)
RS = B * TB          # 1024 rows owned per core

KT = DIM // 128      # 8 contraction tiles (dim / c)
ST = T // 128        # 16 s-tiles of the AFT contraction
SP = ST // 2         # 8 DoubleRow s-pairs
TC2 = T // 512       # 4 tau-chunks of 512
RC = RS // 512       # 2 row-chunks of 512
WSCALE = 32.0        # host scales Wk by this to keep fp8 weights ~N(0,1)
KSHIFT = 2.0         # store exp(k - KSHIFT); cancels in weighted/denom

TRACE = False        # set by test.py for profiling runs


def _install_ntff_hook():
    """The agent image's antenv lacks axon_hooks; recreate it so
    run_bass_kernel_spmd(trace=True) can capture NTFF profiles."""
    import types

    try:
        from antenv.axon_hooks import get_axon_ntff_profile_hook  # noqa: F401
        return
    except ImportError:
        pass
    import antenv

    mod = types.ModuleType("antenv.axon_hooks")
    _h = [None]
    mod.set_axon_ntff_profile_hook = lambda h: _h.__setitem__(0, h)
    mod.get_axon_ntff_profile_hook = lambda: _h[0]
    sys.modules["antenv.axon_hooks"] = mod
    antenv.axon_hooks = mod
    from trn_agent_boot.trn_boot import _ntff_profile_via_ctypes

    mod.set_axon_ntff_profile_hook(
        _ntff_profile_via_ctypes("/opt/axon/libaxon_pjrt.so")
    )


def _build():
    import concourse.bacc as bacc
    import concourse.tile as tile
    import concourse.mybir as mybir

    f32 = mybir.dt.float32
    bf16 = mybir.dt.bfloat16
    fp8 = mybir.dt.float8e4
    AF = mybir.ActivationFunctionType
    ALU = mybir.AluOpType
    DR = mybir.MatmulPerfMode.DoubleRow

    nc = bacc.Bacc("TRN2", debug=False, num_devices=NCORES)

    xT = nc.dram_tensor("xT", [128, KT * RS], bf16, kind="ExternalInput")
    xT8 = nc.dram_tensor("xT8", [128, KT * RS], fp8, kind="ExternalInput")
    wqT = nc.dram_tensor("wqT", [128, KT * DIM], bf16, kind="ExternalInput")
    wkT8 = nc.dram_tensor("wkT8", [128, KT * DIM], fp8, kind="ExternalInput")
    wvT = nc.dram_tensor("wvT", [128, KT * DIM], bf16, kind="ExternalInput")
    woT = nc.dram_tensor("woT", [128, KT * DIM], bf16, kind="ExternalInput")
    bq = nc.dram_tensor("bq", [128, KT], f32, kind="ExternalInput")
    bk = nc.dram_tensor("bk", [128, KT], f32, kind="ExternalInput")
    bv = nc.dram_tensor("bv", [128, KT], f32, kind="ExternalInput")
    bo = nc.dram_tensor("bo", [128, KT], f32, kind="ExternalInput")
    # host-precomputed exp(wbias.T) in fp8: [p, st*T + tau], s = st*128+p
    ewtT = nc.dram_tensor("ewtT", [128, ST * T], fp8, kind="ExternalInput")
    ident = nc.dram_tensor("ident", [128, 128], fp8, kind="ExternalInput")
    out = nc.dram_tensor("out", [DIM, RS], f32, kind="ExternalOutput")

    # [c, row] store free-layout: block j (=c//128) at free j*RS + row.
    # AFT view of rows [r0, r0+n): [128(delta), n, 8] with tau = r*8 + j.
    def aft_view(store, r0, n):
        return store.rearrange("p (j r) -> p j r", j=KT)[
            :, :, r0 : r0 + n
        ].transpose([0, 2, 1])

    with tile.TileContext(nc) as tc:
      with (
        tc.tile_pool(name="const", bufs=1) as constp,
        tc.tile_pool(name="pers", bufs=1) as pers,
      ):
        # ---- persistent stores (per-partition bytes in comments) ----
        sq_sb = pers.tile([128, KT * RS], bf16, tag="sq")    # 16K sigmoid(q)->y
        v_tau = pers.tile([128, B * T], f32, tag="v")        # 32K [delta,b*T+tau]
        # exp(k-KSHIFT) in AFT-flat layout [delta(p), b*T + tau] (tau=r*8+j)
        ek_aft = pers.tile([128, B * T], fp8, tag="ek")      # 8K
        ewt_all = pers.tile([128, ST * T], fp8, tag="ewt")   # 32K exp(wbT)
        eks_sb = pers.tile([128, B * T], fp8, tag="eks")     # 8K  [s, b,st,delta]
        wsum = pers.tile([128, T], f32, tag="wsum")          # 8K
        den = pers.tile([128, T], f32, tag="den")            # 8K
        xts = pers.tile([128, KT * RS], bf16, tag="xts")     # 16K
        xts8 = pers.tile([128, KT * RS], fp8, tag="xts8")    # 8K
        wq_sb = pers.tile([128, KT * DIM], bf16, tag="wq")   # 16K
        wk8_sb = pers.tile([128, KT * DIM], fp8, tag="wk8")  # 8K
        wv_sb = pers.tile([128, KT * DIM], bf16, tag="wv")   # 16K

        # ---- t=0 DMA posts ----
        # sync (HW queue): K operands first, finely split so the first K
        # matmul starts after ~384KB, then V, then Q.
        half = KT * RS // 2
        quarter = half // 2
        nc.sync.dma_start(out=wk8_sb[:, : KT * 128],
                          in_=wkT8[:, : KT * 128])
        nc.sync.dma_start(out=xts8[:, :quarter], in_=xT8[:, :quarter])
        nc.sync.dma_start(out=xts8[:, quarter:half],
                          in_=xT8[:, quarter:half])
        nc.sync.dma_start(out=wk8_sb[:, KT * 128 :], in_=wkT8[:, KT * 128 :])
        nc.sync.dma_start(out=xts8[:, half:], in_=xT8[:, half:])
        nc.sync.dma_start(out=xts, in_=xT[:])
        nc.sync.dma_start(out=wv_sb, in_=wvT[:])
        nc.sync.dma_start(out=wq_sb, in_=wqT[:])
        # exp(wbias) fp8 behind the stage-1 operands on the same queue: it
        # is not needed until the numer matmuls (~75us), and a parallel
        # queue would steal wire bandwidth from the critical K loads.
        for q4 in range(4):
            csz = ST * T // 4
            nc.sync.dma_start(
                out=ewt_all[:, q4 * csz : (q4 + 1) * csz],
                in_=ewtT[:, q4 * csz : (q4 + 1) * csz],
            )
        # scalar (HW queue): small constants, first thing it does.
        id_sb = constp.tile([128, 128], fp8, tag="id")
        nc.scalar.dma_start(out=id_sb, in_=ident[:])
        bias_sb = {}
        for nm, tsr in [("bq", bq), ("bk", bk), ("bv", bv), ("bo", bo)]:
            t_ = constp.tile([128, KT], f32, tag=nm, name=f"b_{nm}")
            nc.scalar.dma_start(out=t_, in_=tsr[:])
            bias_sb[nm] = t_

        ew4 = ewt_all.rearrange("p (st t) -> p st t", st=ST)
        eks4 = eks_sb.rearrange("p (b st d) -> p b st d", b=B, st=ST)
        ek4 = ek_aft.rearrange("p (b r j) -> p b r j", b=B, j=KT)
        xt4 = xts.rearrange("p (rc kt n) -> p rc kt n", rc=RC, kt=KT)
        xt84 = xts8.rearrange("p (rc kt n) -> p rc kt n", rc=RC, kt=KT)
        wq4 = wq_sb.rearrange("p (j kt d) -> p j kt d", j=KT, kt=KT)
        wk84 = wk8_sb.rearrange("p (j kt d) -> p j kt d", j=KT, kt=KT)
        wv4 = wv_sb.rearrange("p (j kt d) -> p j kt d", j=KT, kt=KT)
        v4 = v_tau.rearrange("p (b t j) -> p b t j", b=B, j=KT)

        with (
            tc.tile_pool(name="s3sb", bufs=1) as s3p,
            tc.tile_pool(name="s2f", bufs=1) as s2f,
            tc.tile_pool(name="qkvps", bufs=1, space="PSUM") as ps1,
        ):
            # ---------------- K projection (fp8 DoubleRow) --------------
            for j in range(KT):
                for rc in range(RC):
                    psum = ps1.tile([128, 512], f32, tag="qkv", bufs=3,
                                    name=f"ps_k_{j}_{rc}")
                    for g in range(KT // 2):
                        nc.tensor.matmul(
                            psum,
                            wk84[:, j, 2 * g : 2 * g + 2, :],
                            xt84[:, rc, 2 * g : 2 * g + 2, :],
                            start=(g == 0),
                            stop=(g == KT // 2 - 1),
                            perf_mode=DR,
                        )
                    # exp((psum/WSCALE) + bk - KSHIFT) -> fp8 contiguous on
                    # scalar, then vector scatters into the AFT-flat layout
                    # (strided writes are ~2x slower; splitting engines
                    # keeps the K psum recycling off the scalar's back)
                    ekc = s2f.tile([128, 512], fp8, tag="ekc", bufs=3,
                                   name=f"ekc_{j}_{rc}")
                    nc.scalar.activation(
                        out=ekc, in_=psum, func=AF.Exp,
                        bias=bias_sb["bk"][:, j : j + 1],
                        scale=1.0 / WSCALE,
                    )
                    nc.vector.tensor_copy(
                        out=ek4[:, rc * 2 : (rc + 1) * 2, :, j],
                        in_=ekc.rearrange("p (b r) -> p b r", b=2),
                    )

            # ---------------- V projection (bf16) -----------------------
            for j in range(KT):
                for rc in range(RC):
                    psum = ps1.tile([128, 512], f32, tag="qkv", bufs=3,
                                    name=f"ps_v_{j}_{rc}")
                    for kt in range(KT):
                        nc.tensor.matmul(
                            psum,
                            wv4[:, j, kt, :],
                            xt4[:, rc, kt, :],
                            start=(kt == 0),
                            stop=(kt == KT - 1),
                        )
                    nc.vector.tensor_scalar_add(
                        out=v4[:, rc * 2 : (rc + 1) * 2, :, j],
                        in0=psum.rearrange("p (b t) -> p b t", b=2),
                        scalar1=bias_sb["bv"][:, j : j + 1],
                    )

            # -------- transposes: ek_aft -> eks (fp8, step-2 psum) ------
            # 4 transposes batched per psum tile -> one vector evac per 4.
            with tc.tile_pool(name="trps", bufs=1, space="PSUM") as pst:
                for grp in range(B * ST // 4):
                    b, st0 = grp // 4, (grp % 4) * 4
                    tp = pst.tile([128, 1024], fp8, tag="tr", bufs=3,
                                  name=f"tp_{b}_{st0}")
                    for q in range(4):
                        tq = tp[:, q * 256 : (q + 1) * 256].rearrange(
                            "p (n two) -> p n two", two=2)[:, :, 0]
                        nc.tensor.transpose(
                            tq,
                            ek_aft[:, (b * ST + st0 + q) * 128 :
                                   (b * ST + st0 + q + 1) * 128],
                            id_sb,
                        )
                    nc.vector.tensor_copy(
                        out=eks4[:, b, st0 : st0 + 4, :],
                        in_=tp.rearrange("p (f n two) -> p f n two",
                                         f=4, two=2)[:, :, :, 0],
                    )

            # prefetch wo tiles on sync queue (landing during stage 2)
            wod_tiles = []
            for dt_ in range(KT):
                wod = s3p.tile([128, KT * 128], bf16, tag="wod", bufs=8,
                               name=f"wod_{dt_}")
                nc.sync.dma_start(
                    out=wod, in_=woT[:, dt_ * KT * 128 : (dt_ + 1) * KT * 128]
                )
                wod_tiles.append(wod)

            # ---- AFT numer (fp8 DoubleRow) + chain, Q interleaved ------
            # Q matmuls are emitted between numer chunks so the PE stays
            # busy while each chunk's vector chain drains the nps banks.
            with tc.tile_pool(name="s2ps", bufs=1, space="PSUM") as ps2:
                for tc2 in range(TC2):
                    tsl = slice(tc2 * 512, (tc2 + 1) * 512)
                    nps = [ps2.tile([128, 512], f32, tag="np", bufs=5,
                                    name=f"np_{tc2}_{b}") for b in range(B)]
                    for sp in range(SP):
                        for b in range(B):
                            nc.tensor.matmul(
                                nps[b],
                                eks4[:, b, 2 * sp : 2 * sp + 2, :],
                                ew4[:, 2 * sp : 2 * sp + 2, tsl],
                                start=(sp == 0),
                                stop=(sp == SP - 1),
                                perf_mode=DR,
                            )
                    for b in range(B):
                        vview = v_tau[:, b * T + tc2 * 512 :
                                      b * T + (tc2 + 1) * 512]
                        if b == 0:
                            nc.vector.tensor_tensor(
                                out=wsum[:, tsl], in0=nps[b], in1=vview,
                                op=ALU.mult,
                            )
                            nc.vector.tensor_copy(out=den[:, tsl], in_=nps[b])
                        else:
                            nv = s2f.tile([128, 512], f32, tag="nv", bufs=2,
                                          name=f"nv_{tc2}_{b}")
                            nc.vector.tensor_tensor(
                                out=nv, in0=nps[b], in1=vview, op=ALU.mult,
                            )
                            nc.vector.tensor_add(
                                out=wsum[:, tsl], in0=wsum[:, tsl], in1=nv
                            )
                            nc.vector.tensor_add(
                                out=den[:, tsl], in0=den[:, tsl], in1=nps[b]
                            )
                    rec = s2f.tile([128, 512], f32, tag="rec", bufs=2,
                                   name=f"rec_{tc2}")
                    nc.vector.reciprocal_approx_fast(out=rec, in_=den[:, tsl])
                    nc.vector.tensor_tensor(out=wsum[:, tsl], in0=wsum[:, tsl],
                                            in1=rec, op=ALU.mult)
                    # 4 Q psums per numer chunk (j = 2*tc2, 2*tc2+1)
                    for j in (2 * tc2, 2 * tc2 + 1):
                        for rc in range(RC):
                            psum = ps1.tile([128, 512], f32, tag="qkv",
                                            bufs=3, name=f"ps_q_{j}_{rc}")
                            for kt in range(KT):
                                nc.tensor.matmul(
                                    psum,
                                    wq4[:, j, kt, :],
                                    xt4[:, rc, kt, :],
                                    start=(kt == 0),
                                    stop=(kt == KT - 1),
                                )
                            nc.scalar.activation(
                                out=sq_sb[:, j * RS + rc * 512 :
                                          j * RS + (rc + 1) * 512],
                                in_=psum, func=AF.Sigmoid,
                                bias=bias_sb["bq"][:, j : j + 1],
                            )

            # y = sigmoid(q) * weighted, in the [c,row] layout: for block j,
            # in1[row] = wsum[p, t*8 + j] (t = row % 256), contiguous out.
            for j in range(KT):
                wsl = wsum.rearrange("p (t j) -> p t j", j=KT)[:, :, j]
                for rc in range(RC):
                    for u in range(2):
                        o0 = j * RS + rc * 512 + u * 256
                        nc.vector.tensor_tensor(
                            out=sq_sb[:, o0 : o0 + 256],
                            in0=sq_sb[:, o0 : o0 + 256],
                            in1=wsl, op=ALU.mult,
                        )

            # ------------ out projection (bf16), wo resident ------------
            # evac alternates scalar/vector and the out-DMA alternates
            # sync/scalar queues so no single engine paces the drain.
            with tc.tile_pool(name="s3ps", bufs=1, space="PSUM") as ps3:
                for dt_ in range(KT):
                    for rc in range(RC):
                        i3 = dt_ * RC + rc
                        rsl = slice(rc * 512, (rc + 1) * 512)
                        pso = ps3.tile([128, 512], f32, tag="o", bufs=5,
                                       name=f"pso_{rc}_{dt_}")
                        for j in range(KT):
                            nc.tensor.matmul(
                                pso,
                                wod_tiles[dt_][:, j * 128 : (j + 1) * 128],
                                sq_sb[:, j * RS + rc * 512 :
                                      j * RS + (rc + 1) * 512],
                                start=(j == 0),
                                stop=(j == KT - 1),
                            )
                        osb = s3p.tile([128, 512], f32, tag="ot", bufs=3,
                                       name=f"osb_{rc}_{dt_}")
                        if i3 % 2 == 0:
                            nc.scalar.activation(
                                out=osb, in_=pso, func=AF.Identity,
                                bias=bias_sb["bo"][:, dt_ : dt_ + 1],
                            )
                            nc.sync.dma_start(
                                out=out[dt_ * 128 : (dt_ + 1) * 128, rsl],
                                in_=osb,
                            )
                        else:
                            nc.vector.tensor_scalar_add(
                                out=osb, in0=pso,
                                scalar1=bias_sb["bo"][:, dt_ : dt_ + 1],
                            )
                            nc.scalar.dma_start(
                                out=out[dt_ * 128 : (dt_ + 1) * 128, rsl],
                                in_=osb,
                            )

    nc.compile()
    return nc


_NC_CACHE = None


def make_in_maps(x, Wq, bq, Wk, bk, Wv, bv, wbias, Wo, bo):
    import ml_dtypes

    f = np.float32
    bf = ml_dtypes.bfloat16
    f8 = ml_dtypes.float8_e4m3
    x = np.asarray(x, f)
    Wq, Wk, Wv, Wo = (np.asarray(a, f) for a in (Wq, Wk, Wv, Wo))
    bq, bk, bv, bo = (np.asarray(a, f) for a in (bq, bk, bv, bo))
    wbias = np.asarray(wbias, f)

    x2 = x.reshape(B * T, DIM)

    def tile_w(W):
        # host[p, X*1024 + Y*128 + d] = W[X*128+d, Y*128+p]
        return np.ascontiguousarray(
            W.reshape(KT, 128, KT, 128).transpose(3, 0, 2, 1).reshape(
                128, KT * KT * 128)
        )

    wqT = tile_w(Wq).astype(bf)
    wkT8 = tile_w(Wk * WSCALE).astype(f8)
    wvT = tile_w(Wv).astype(bf)
    woT = tile_w(Wo).astype(bf)
    id_np = np.eye(128, dtype=f).astype(f8)
    bqc = np.ascontiguousarray(bq.reshape(KT, 128).T)
    bkc = np.ascontiguousarray(bk.reshape(KT, 128).T) - KSHIFT
    bvc = np.ascontiguousarray(bv.reshape(KT, 128).T)
    boc = np.ascontiguousarray(bo.reshape(KT, 128).T)

    in_maps = []
    for c in range(NCORES):
        rows = np.concatenate(
            [x2[b * T + c * TB : b * T + (c + 1) * TB] for b in range(B)]
        )  # [RS, DIM], row = b*TB + t_loc
        xtiled = np.ascontiguousarray(
            rows.T.reshape(KT, 128, RC, 512).transpose(1, 2, 0, 3)
            .reshape(128, KT * RS))
        # ewtT[p, st*T + tau] = exp(wbias[c][tau, st*128+p])
        ewt = np.ascontiguousarray(
            np.exp(wbias[c].T).reshape(ST, 128, T).transpose(1, 0, 2)
            .reshape(128, ST * T)).astype(f8)
        in_maps.append({
            "xT": xtiled.astype(bf),
            "xT8": xtiled.astype(f8),
            "wqT": wqT, "wkT8": wkT8, "wvT": wvT, "woT": woT,
            "bq": bqc, "bk": bkc, "bv": bvc, "bo": boc,
            "ewtT": ewt,
            "ident": id_np,
        })
    return in_maps


def kernel(x, Wq, bq, Wk, bk, Wv, bv, wbias, Wo, bo):
    global _NC_CACHE
    from concourse import bass_utils

    in_maps = make_in_maps(x, Wq, bq, Wk, bk, Wv, bv, wbias, Wo, bo)

    if TRACE:
        _install_ntff_hook()
    if _NC_CACHE is None:
        _NC_CACHE = _build()
    nc = _NC_CACHE

    res = bass_utils.run_bass_kernel_spmd(
        nc, in_maps, core_ids=list(range(NCORES)), trace=TRACE
    )
    f = np.float32
    outf = np.empty((B * T, DIM), f)
    for c in range(NCORES):
        blk = res.results[c]["out"].T  # [RS, DIM], row = b*TB + t_loc
        for b in range(B):
            outf[b * T + c * TB : b * T + (c + 1) * TB] = (
                blk[b * TB : (b + 1) * TB]
            )
    if TRACE:
        kernel.last_exec_time_ns = res.exec_time_ns
        kernel.last_results = res
    return outf.reshape(B, T, DIM)
